# revision 1
# baseline (speedup 1.0000x reference)
"""Trainium2 Bass kernel for nn_AffNet (affinity network).

Reference computation:
    X_emb = X @ W                               # [N, E]
    aff_h = (Z_h @ X_emb^T) / (|X_emb| |Z_h|)   # cosine, [H, N, N]
    aff   = max_h aff_h                          # [N, N]
    aff   = (aff + aff^T) / 2                    # symmetrize
    aff   = (aff + 1) / 2                        # [0, 1]
    aff   = aff ** beta

Device strategy (8 NeuronCores, symmetric block-pair parallel):
  The output is symmetric by construction, so the 16x16 grid of 512x512
  blocks has 120 off-diagonal pairs {(i,j),(j,i)} + 16 diagonal blocks.
  Each core gets 15 pairs + 2 diagonal blocks (exactly 1/8 of the work).
  For a pair, the core computes the pooled block once:
      direct[m, n] = maxP'[m, n] + maxQ'[m, n] + 0.5
  where P'_h = Z''_h[rows_i] . X'[cols_j] and Q'_h = X'[rows_i] . Z''_h[cols_j]
  (normalized operands; x1/4 folded into Z''), which equals
  ((maxP + maxP^T)/2 + 1)/2 on that block, and gets the mirror block
  (j, i) as a TensorE transpose of the direct block — no recompute.
  Diagonal blocks are symmetric by construction and need no mirror.

  SPMD: all cores run the identical program over 17 fixed "slots"; the
  host permutes input columns per core (row-block / col-block copies)
  and scatters the 32 output blocks into the final matrix (adding the
  final +0.5 and upcasting the bf16 device output to fp32 there).

  Per [128, 512] output tile: 8 matmuls (4 P heads + 4 Q heads) into
  eight PSUM banks (heads 1,3 through two single-bank ScalarE-evacuated
  chains; heads 2,4 into two 2-bank tiles consumed by VectorE); ScalarE
  evacuates 4 blocks to bf16, VectorE does two fused L1 maxes (fp32 PSUM
  x bf16 SBUF), one strided bf16 2x L2 max, and a bf16 2x final add.
  Mirror blocks flow through two dedicated transpose PSUM banks.
  Engine balance (cost model, per core): DVE ~210us (bottleneck, 92%
  busy), ScalarE ~188us, PE ~125us, DMA ~108us -> ~228us total.
"""

import numpy as np

N_NODES = 8192
N_FEATURES = 512
EMB = 128
N_HEADS = 4
EPS = 1e-6
N_CORES = 8
BLK = 512                     # symmetric block size
N_BLK = N_NODES // BLK        # 16 row/col blocks
M_CHUNK = 128                 # rows per matmul (PSUM partitions)
N_PAIRS = 15                  # off-diagonal pairs per core
N_DIAG = 2                    # diagonal blocks per core
N_SLOTS = N_PAIRS + N_DIAG    # 17
SLOT_COLS = N_SLOTS * BLK     # 8704

_CACHE = {}
LAST_RESULT = None


def _assignments():
    """Global block->core assignment, identical on every call."""
    pairs = [(i, j) for i in range(N_BLK) for j in range(i + 1, N_BLK)]
    diags = [(i, i) for i in range(N_BLK)]
    per_core = []
    for c in range(N_CORES):
        my = pairs[c::N_CORES] + diags[c::N_CORES]
        assert len(my) == N_SLOTS
        per_core.append(my)
    return per_core


def _split_multi_waits(nc, limit=1):
    """The walrus build in this environment encodes at most one semaphore
    wait per instruction ("Too many sync wait commands" otherwise), while
    Tile attaches several. Hoist extra waits onto same-engine NOPs inserted
    immediately before the instruction (waits still execute before it)."""
    import concourse.mybir as mybir

    for f in nc.m.functions:
        for bb in f.blocks:
            il = bb.instructions  # live list backing the block
            idx = 0
            while idx < len(il):
                inst = il[idx]
                si = inst.sync_info
                waits = list(si.on_wait) if si is not None and si.on_wait else []
                if len(waits) > limit:
                    ups = list(si.on_update) if si.on_update else []
                    inst.sync_info = mybir.SyncInfo(
                        on_wait=waits[:limit], on_update=ups
                    )
                    eng = nc.engines[inst.engine]
                    pos = idx
                    for j in range(limit, len(waits), limit):
                        nbi = eng.nop()
                        ninst = nbi.ins
                        # nop() appended itself to the current bb; detach it
                        removed = False
                        for f2 in nc.m.functions:
                            for bb2 in f2.blocks:
                                l2 = bb2.instructions
                                if l2 and l2[-1].name == ninst.name:
                                    l2.pop()
                                    removed = True
                                    break
                            if removed:
                                break
                        assert removed, "could not detach helper nop"
                        ninst.sync_info = mybir.SyncInfo(
                            on_wait=waits[j : j + limit], on_update=[]
                        )
                        il.insert(pos, ninst)
                        pos += 1
                        idx += 1
                idx += 1


def _build_program():
    import concourse.bass as bass
    import concourse.mybir as mybir
    import concourse.tile as tile
    from concourse.masks import make_identity

    nc = bass.Bass("TRN2", target_bir_lowering=False, debug=False)

    bf16 = mybir.dt.bfloat16
    f32 = mybir.dt.float32
    # Per-core slot-major operands (host packs [slot][E, BLK] slices)
    xr = nc.dram_tensor("xr", [N_PAIRS, EMB, BLK], bf16, kind="ExternalInput")
    xc = nc.dram_tensor("xc", [N_SLOTS, EMB, BLK], bf16, kind="ExternalInput")
    zr = nc.dram_tensor("zr", [N_HEADS, N_SLOTS, EMB, BLK], bf16,
                        kind="ExternalInput")
    zc = nc.dram_tensor("zc", [N_HEADS, N_PAIRS, EMB, BLK], bf16,
                        kind="ExternalInput")
    # pair slots ship {maxP, maxQ} interleaved per row-chunk; diagonal
    # slots ship maxP only. The host finishes the elementwise epilogue
    # (maxP + maxQ + 0.5, upcast, and the mirror/diagonal transposes)
    # during output assembly.
    outd = nc.dram_tensor("outd", [N_PAIRS, BLK, 2, BLK], bf16,
                          kind="ExternalOutput")
    outdd = nc.dram_tensor("outdd", [N_DIAG, BLK, BLK], bf16,
                           kind="ExternalOutput")

    n_m = BLK // M_CHUNK  # 4 m-chunks per block

    with tile.TileContext(nc) as tc:
        with (
            tc.tile_pool(name="weights", bufs=1) as wpool,
            tc.tile_pool(name="psum", bufs=1, space="PSUM") as ppool,
            tc.tile_pool(name="work", bufs=2) as spool,
        ):
            for s in range(N_SLOTS):
                is_diag = s >= N_PAIRS
                # per-slot input tiles (multi-buffered so prefetch
                # overlaps); xc + zr first: the first matmuls need them.
                # Diagonal slots never touch xr/zc, so skip those loads.
                xc_s = spool.tile([EMB, BLK], bf16, tag="xc", bufs=4,
                                  name=f"xc_{s}")
                nc.sync.dma_start(out=xc_s, in_=xc[s])
                zr_s, zc_s = [], []
                for h in range(N_HEADS):
                    t = spool.tile([EMB, BLK], bf16, tag=f"zr{h}", bufs=4,
                                   name=f"zr{h}_{s}")
                    nc.sync.dma_start(out=t, in_=zr[h, s])
                    zr_s.append(t)
                if not is_diag:
                    xr_s = spool.tile([EMB, BLK], bf16, tag="xr", bufs=4,
                                      name=f"xr_{s}")
                    nc.sync.dma_start(out=xr_s, in_=xr[s])
                    for h in range(N_HEADS):
                        t = spool.tile([EMB, BLK], bf16, tag=f"zc{h}", bufs=4,
                                       name=f"zc{h}_{s}")
                        nc.sync.dma_start(out=t, in_=zc[h, s])
                        zc_s.append(t)

                if is_diag:
                    # Diagonal block: P[A,A] and Q[A,A] are transposes of
                    # each other, so compute only the P matmuls, pool the 4
                    # heads, and finish with out = maxP + maxP^T via
                    # TensorE transposes. Half the matmuls and pooling.
                    dmx = []  # pooled maxP tiles, [128, 4, 128] bf16
                    for m in range(n_m):
                        msl = slice(m * M_CHUNK, (m + 1) * M_CHUNK)
                        ap1 = ppool.tile([M_CHUNK, BLK], f32, tag="ap_a",
                                         name=f"dap1_{s}_{m}")
                        b1 = ppool.tile([M_CHUNK, 2, BLK], f32, tag="b1",
                                        name=f"db1_{s}_{m}")
                        nc.tensor.matmul(ap1, zr_s[0][:, msl], xc_s,
                                         start=True, stop=True)
                        nc.tensor.matmul(b1[:, 0], zr_s[1][:, msl], xc_s,
                                         start=True, stop=True)
                        ea = spool.tile([M_CHUNK, 4, BLK], bf16, tag="ea",
                                        bufs=4, name=f"dea_{s}_{m}")
                        nc.scalar.copy(ea[:, 0], ap1)
                        ap3 = ppool.tile([M_CHUNK, BLK], f32, tag="ap_b",
                                         name=f"dap3_{s}_{m}")
                        nc.tensor.matmul(b1[:, 1], zr_s[3][:, msl], xc_s,
                                         start=True, stop=True)
                        nc.tensor.matmul(ap3, zr_s[2][:, msl], xc_s,
                                         start=True, stop=True)
                        nc.scalar.copy(ea[:, 1], ap3)
                        l1 = spool.tile([M_CHUNK, 2, BLK], bf16, tag="l1",
                                        bufs=3, name=f"dl1_{s}_{m}")
                        nc.vector.tensor_max(l1, b1, ea[:, 0:2])
                        dm = spool.tile([M_CHUNK, 4, M_CHUNK], bf16,
                                        tag="dmx", bufs=5, name=f"dmx_{s}_{m}")
                        nc.vector.tensor_max(dm, l1[:, 0], l1[:, 1])
                        dmx.append(dm)
                    for m in range(n_m):
                        nc.gpsimd.dma_start(
                            out=outdd[s - N_PAIRS,
                                      m * M_CHUNK:(m + 1) * M_CHUNK, :],
                            in_=dmx[m],
                        )
                    continue

                l1d = None
                for m in range(n_m):
                    msl = slice(m * M_CHUNK, (m + 1) * M_CHUNK)
                    half4 = 4 * (m % 2)
                    # PSUM bank map (8 banks):
                    #   ap_a {P1}, ap_b {P3}, aq {Q1 then Q3} -- ScalarE
                    #     evacuates these fast (short independent chains).
                    #   b1 {P2,Q2}, b2 {P4,Q4} -- freed by the two fused
                    #     VectorE L1 maxes (the pipeline bottleneck).
                    #   tp -- dedicated transpose bank, keeping the mirror
                    #     path off the matmul critical path.
                    ap1 = ppool.tile([M_CHUNK, BLK], f32, tag="ap_a",
                                     name=f"ap1_{s}_{m}")
                    aq1 = ppool.tile([M_CHUNK, BLK], f32, tag="aq",
                                     name=f"aq1_{s}_{m}")
                    b1 = ppool.tile([M_CHUNK, 2, BLK], f32, tag="b1",
                                    name=f"b1_{s}_{m}")
                    b2 = ppool.tile([M_CHUNK, 2, BLK], f32, tag="b2",
                                    name=f"b2_{s}_{m}")
                    nc.tensor.matmul(ap1, zr_s[0][:, msl], xc_s,
                                     start=True, stop=True)
                    nc.tensor.matmul(aq1, xr_s[:, msl], zc_s[0],
                                     start=True, stop=True)
                    nc.tensor.matmul(b1[:, 0], zr_s[1][:, msl], xc_s,
                                     start=True, stop=True)
                    nc.tensor.matmul(b1[:, 1], xr_s[:, msl], zc_s[1],
                                     start=True, stop=True)
                    # ScalarE: evacuate A-blocks fp32 -> bf16 SBUF as they
                    # land; two independent single-bank chains (P and Q)
                    # ea layout: {eP1, eP3, eQ1, eQ3}
                    ea = spool.tile([M_CHUNK, 4, BLK], bf16, tag="ea", bufs=4)
                    nc.scalar.copy(ea[:, 0], ap1)
                    nc.scalar.copy(ea[:, 2], aq1)
                    ap3 = ppool.tile([M_CHUNK, BLK], f32, tag="ap_b",
                                     name=f"ap3_{s}_{m}")
                    aq3 = ppool.tile([M_CHUNK, BLK], f32, tag="aq",
                                     name=f"aq3_{s}_{m}")
                    nc.tensor.matmul(b2[:, 0], zr_s[3][:, msl], xc_s,
                                     start=True, stop=True)
                    nc.tensor.matmul(b2[:, 1], xr_s[:, msl], zc_s[3],
                                     start=True, stop=True)
                    nc.tensor.matmul(ap3, zr_s[2][:, msl], xc_s,
                                     start=True, stop=True)
                    nc.tensor.matmul(aq3, xr_s[:, msl], zc_s[2],
                                     start=True, stop=True)
                    nc.scalar.copy(ea[:, 1], ap3)
                    nc.scalar.copy(ea[:, 3], aq3)
                    # VectorE L1: l1 = {m12P, m34P, m12Q, m34Q} per tile,
                    # two tiles sharing one l1 tile so the SBUF-side
                    # combines run once per tile pair at full width
                    if m % 2 == 0:
                        l1d = spool.tile([M_CHUNK, 8, BLK], bf16, tag="l1",
                                         bufs=3, name=f"l1_{s}_{m}")
                    nc.vector.tensor_max(l1d[:, half4 + 0:half4 + 4:2],
                                         b1, ea[:, 0:4:2])
                    nc.vector.tensor_max(l1d[:, half4 + 1:half4 + 4:2],
                                         b2, ea[:, 1:4:2])
                    if m % 2 == 1:
                        # L2 (both tiles): {maxP0, maxQ0, maxP1, maxQ1};
                        # shipped as-is, host adds them during assembly
                        l2 = spool.tile([M_CHUNK, 4, BLK], bf16, tag="l2",
                                        bufs=4, name=f"l2_{s}_{m}")
                        nc.vector.tensor_max(l2, l1d[:, 0:8:2], l1d[:, 1:8:2])
                        nc.gpsimd.dma_start(
                            out=outd[s, (m - 1) * M_CHUNK:m * M_CHUNK, :, :],
                            in_=l2[:, 0:2],
                        )
                        nc.gpsimd.dma_start(
                            out=outd[s, m * M_CHUNK:(m + 1) * M_CHUNK, :, :],
                            in_=l2[:, 2:4],
                        )


    _split_multi_waits(nc)
    return nc


def kernel(X, W, Z, beta):
    global LAST_RESULT
    import ml_dtypes
    from concourse.bass_utils import run_bass_kernel_spmd

    X = np.asarray(X, dtype=np.float32)
    W = np.asarray(W, dtype=np.float32)
    Z = np.asarray(Z, dtype=np.float32)
    beta_f = float(np.asarray(beta))

    # Host: normalized, transposed, bf16 operands
    X_emb = X @ W                                            # [N, E] fp32
    Xn = np.sqrt(np.sum(X_emb * X_emb, axis=-1))             # [N]
    Zn = np.sqrt(np.sum(Z * Z, axis=-1))                     # [H, N]
    Xp = X_emb / (Xn[:, None] + EPS)                         # [N, E]
    Zp = Z / (Zn[:, :, None] + EPS) * 0.25                   # [H, N, E]
    bf16 = ml_dtypes.bfloat16
    XpT = np.ascontiguousarray(Xp.T).astype(bf16)            # [E, N]
    ZpT = np.ascontiguousarray(Zp.transpose(0, 2, 1)).astype(bf16)  # [H, E, N]

    if "nc" not in _CACHE:
        _CACHE["nc"] = _build_program()
    nc = _CACHE["nc"]

    assign = _assignments()
    in_maps = []
    for c in range(N_CORES):
        blocks = assign[c]
        ridx = np.concatenate(
            [np.arange(i * BLK, (i + 1) * BLK) for (i, j) in blocks]
        )
        cidx = np.concatenate(
            [np.arange(j * BLK, (j + 1) * BLK) for (i, j) in blocks]
        )
        def slotize_x(a):  # [E, 17*BLK] -> [17, E, BLK]
            return np.ascontiguousarray(
                a.reshape(EMB, N_SLOTS, BLK).transpose(1, 0, 2)
            )

        def slotize_z(a):  # [H, E, 17*BLK] -> [H, 17, E, BLK]
            return np.ascontiguousarray(
                a.reshape(N_HEADS, EMB, N_SLOTS, BLK).transpose(0, 2, 1, 3)
            )

        in_maps.append(
            {
                "xr": slotize_x(XpT[:, ridx])[:N_PAIRS],
                "xc": slotize_x(XpT[:, cidx]),
                "zr": slotize_z(ZpT[:, :, ridx]),
                "zc": slotize_z(ZpT[:, :, cidx])[:, :N_PAIRS],
            }
        )

    res = None
    for attempt in range(3):
        try:
            res = run_bass_kernel_spmd(nc, in_maps, list(range(N_CORES)))
            break
        except Exception:
            if attempt == 2:
                raise
    LAST_RESULT = res

    outp = np.empty((N_NODES, N_NODES), dtype=np.float32)
    for c in range(N_CORES):
        blocks = assign[c]
        outd = res.results[c]["outd"]    # [N_PAIRS, BLK, 2, BLK] {maxP,maxQ}
        outdd = res.results[c]["outdd"]  # [N_DIAG, BLK, BLK] maxP
        for s, (i, j) in enumerate(blocks):
            risl = slice(i * BLK, (i + 1) * BLK)
            cjsl = slice(j * BLK, (j + 1) * BLK)
            if i != j:
                blk = outd[s]
                S = blk[:, 0].astype(np.float32)
                S += blk[:, 1]
                S += np.float32(0.5)
                outp[risl, cjsl] = S
                outp[cjsl, risl] = S.T
            else:
                M = outdd[s - N_PAIRS].astype(np.float32)
                M += M.T
                M += np.float32(0.5)
                outp[risl, cjsl] = M

    if beta_f != 1.0:
        outp = np.power(outp, beta_f, dtype=np.float32)
    return outp



# revision 2
# speedup vs baseline: 1.0864x; 1.0864x over previous
"""Trainium2 Bass kernel for nn_AffNet (affinity network) — v3.

Reference computation:
    X_emb = X @ W                               # [N, E]
    aff_h = (Z_h @ X_emb^T) / (|X_emb| |Z_h|)   # cosine, [H, N, N]
    aff   = max_h aff_h                          # [N, N]
    aff   = (aff + aff^T) / 2                    # symmetrize
    aff   = (aff + 1) / 2                        # [0, 1]
    aff   = aff ** beta

Device strategy (8 NeuronCores, output-row parallel):
  Each core computes 2 of the 16 block-rows of the POOLED (pre-symmetrize)
  affinity A = max_h(Zh_hat @ Xh_hat^T): 8 m-chunk rows x 16 col blocks of
  [128, 512] tiles. The device only pools 4 heads -> 2 half-pooled planes;
  the host finishes with A = max(plane0, plane1), then
  out = 0.25*(A + A^T) + 0.5 and ^beta.

  Rationale (cost model): TensorTensor ops allow at most one PSUM operand,
  the Pool engine supports no two-tensor elementwise op at all, and matmul
  PSUM output is fp32-only on TRN2. So PSUM evacuation (ScalarE copies +
  VectorE 1-PSUM maxes) is the hard floor (~262K free-elem units/core
  across Act+DVE). Shipping two bf16 planes per tile instead of one moves
  the final merge to the host, trading idle DMA bandwidth (~102us, under
  the ~145us engine floor) for the DVE L2 work.

  All operands stay resident in SBUF (Xhat^T replicated 16KB/partition,
  Zhat^T own rows 8KB/partition; one DMA each). Outputs accumulate into a
  [128, 16, 2, 512] ship-stripe per m-chunk row -> 8 output DMAs per core.

  Per tile: 4 matmuls (heads) into one 4-bank PSUM quad (bufs=2 => all 8
  banks). Route mix per tile, statically balanced so Act ~143us, DVE
  ~145us (cost model):
    A2 (116 tiles): ScalarE copies banks {0,1} (fused [128,2,512]);
        VectorE does the fused max(banks {2,3}, e01) straight into the
        ship-stripe. Host max merges (b0,b2) and (b1,b3) planes.
    A4 (12 tiles): ScalarE copies all 4 banks; VectorE does one fused
        bf16 2x max into the stripe.
"""

import numpy as np

N_NODES = 8192
N_FEATURES = 512
EMB = 128
N_HEADS = 4
EPS = 1e-6
N_CORES = 8
BLK = 512
N_BLK = N_NODES // BLK          # 16 col blocks
M_CHUNK = 128
BLOCKS_PER_CORE = 2             # block-rows per core
MROWS = BLOCKS_PER_CORE * (BLK // M_CHUNK)   # 8 m-chunk rows per core
ROWS_PER_CORE = BLOCKS_PER_CORE * BLK        # 1024

_CACHE = {}
LAST_RESULT = None


def _a4_cols(q):
    """Columns of m-row q that take the Act-heavy A4 route (12 of 128)."""
    return ()  # all-A2: Act steadily ahead, no burst stalls


def _split_multi_waits(nc, limit=1):
    """The walrus build in this environment encodes at most one semaphore
    wait per instruction ("Too many sync wait commands" otherwise), while
    Tile attaches several. Hoist extra waits onto same-engine NOPs inserted
    immediately before the instruction (waits still execute before it)."""
    import concourse.mybir as mybir

    for f in nc.m.functions:
        for bb in f.blocks:
            il = bb.instructions  # live list backing the block
            idx = 0
            while idx < len(il):
                inst = il[idx]
                si = inst.sync_info
                waits = list(si.on_wait) if si is not None and si.on_wait else []
                if len(waits) > limit:
                    ups = list(si.on_update) if si.on_update else []
                    inst.sync_info = mybir.SyncInfo(
                        on_wait=waits[:limit], on_update=ups
                    )
                    eng = nc.engines[inst.engine]
                    pos = idx
                    for j in range(limit, len(waits), limit):
                        nbi = eng.nop()
                        ninst = nbi.ins
                        # nop() appended itself to the current bb; detach it
                        removed = False
                        for f2 in nc.m.functions:
                            for bb2 in f2.blocks:
                                l2 = bb2.instructions
                                if l2 and l2[-1].name == ninst.name:
                                    l2.pop()
                                    removed = True
                                    break
                            if removed:
                                break
                        assert removed, "could not detach helper nop"
                        ninst.sync_info = mybir.SyncInfo(
                            on_wait=waits[j : j + limit], on_update=[]
                        )
                        il.insert(pos, ninst)
                        pos += 1
                        idx += 1
                idx += 1


def _build_program():
    import concourse.bass as bass
    import concourse.mybir as mybir
    import concourse.tile as tile

    nc = bass.Bass("TRN2", target_bir_lowering=False, debug=False)

    bf16 = mybir.dt.bfloat16
    f32 = mybir.dt.float32

    # Xhat^T full, [E, 16, 512]; Zhat^T own rows, [E, H, 1024]
    xt = nc.dram_tensor("xt", [EMB, N_BLK, BLK], bf16, kind="ExternalInput")
    zt = nc.dram_tensor("zt", [EMB, N_HEADS, ROWS_PER_CORE], bf16,
                        kind="ExternalInput")
    # two half-pooled planes per tile; host merges
    aout = nc.dram_tensor("aout", [MROWS, M_CHUNK, N_BLK, 2, BLK], bf16,
                          kind="ExternalOutput")

    with tile.TileContext(nc) as tc:
        with (
            tc.tile_pool(name="weights", bufs=1) as wpool,
            tc.tile_pool(name="psum", bufs=1, space="PSUM") as ppool,
            tc.tile_pool(name="work", bufs=2) as spool,
        ):
            xt_s = wpool.tile([EMB, N_BLK, BLK], bf16, name="xt_s")
            zt_s = wpool.tile([EMB, N_HEADS, ROWS_PER_CORE], bf16,
                              name="zt_s")
            # Load order tuned for a fast first tile: heads {0,1} of zt and
            # the first xt block arrive first, the rest streams behind.
            nc.sync.dma_start(out=zt_s[:, 0:2], in_=zt[:, 0:2])
            nc.sync.dma_start(out=xt_s[:, 0:1], in_=xt[:, 0:1])
            nc.sync.dma_start(out=zt_s[:, 2:4], in_=zt[:, 2:4])
            for lo, hi in ((1, 4), (4, 10), (10, 16)):
                nc.sync.dma_start(out=xt_s[:, lo:hi], in_=xt[:, lo:hi])

            tiles = [(q, j) for q in range(MROWS) for j in range(N_BLK)]
            stripes = {}
            deferred = []

            def _flush_deferred():
                while deferred:
                    seg_, slot_, a_, b_ = deferred.pop(0)
                    nc.vector.tensor_max(seg_[:, slot_], a_, b_)

            def emit_apr_side(t):
                """Act-pair matmuls + ScalarE evacuation for tile t.
                Emitted one tile AHEAD of the DVE side so the e01 operand
                is ready a full tile before the DVE max needs it."""
                q, j = tiles[t]
                msl = slice(q * M_CHUNK, (q + 1) * M_CHUNK)
                rhs = xt_s[:, j]
                apr = ppool.tile([M_CHUNK, 2, BLK], f32, tag="apr",
                                 bufs=2, name=f"apr_{q}_{j}")
                nc.tensor.matmul(apr[:, 0], zt_s[:, 0, msl], rhs,
                                 start=True, stop=True)
                nc.tensor.matmul(apr[:, 1], zt_s[:, 1, msl], rhs,
                                 start=True, stop=True)
                e01 = spool.tile([M_CHUNK, 2, BLK], bf16, tag="e01",
                                 bufs=6, name=f"e01_{q}_{j}")
                nc.scalar.copy(e01, apr)
                return e01

            def emit_dpr_side(t, e01):
                q, j = tiles[t]
                msl = slice(q * M_CHUNK, (q + 1) * M_CHUNK)
                rhs = xt_s[:, j]
                if j % 4 == 0:
                    # 4-column output segment: ships as soon as its four
                    # tiles are pooled, so output DMA overlaps compute and
                    # the kernel tail only pays one ~3us segment.
                    stripes[q] = spool.tile(
                        [M_CHUNK, 4, 2, BLK], bf16,
                        tag="seg", bufs=8, name=f"seg_{q}_{j // 4}")
                seg = stripes[q]
                dpr = ppool.tile([M_CHUNK, 2, BLK], f32, tag="dpr",
                                 bufs=2, name=f"dpr_{q}_{j}")
                nc.tensor.matmul(dpr[:, 0], zt_s[:, 2, msl], rhs,
                                 start=True, stop=True)
                nc.tensor.matmul(dpr[:, 1], zt_s[:, 3, msl], rhs,
                                 start=True, stop=True)
                if j in _a4_cols(q):
                    # Act-heavy: evacuate the DVE pair too, one fused
                    # bf16 2x max on DVE. The DVE max is DEFERRED one
                    # tile: the Act copy burst (e01 of t+1 then e23 of t)
                    # would otherwise stall the in-order DVE stream.
                    e23 = spool.tile([M_CHUNK, 2, BLK], bf16, tag="e23",
                                     bufs=3, name=f"e23_{q}_{j}")
                    nc.scalar.copy(e23, dpr)
                    deferred.append((seg, j % 4, e01, e23))
                else:
                    # Main route: DVE fused max(dve-pair, e01)
                    nc.vector.tensor_max(seg[:, j % 4], dpr, e01)
                    _flush_deferred()
                if j % 4 == 3:
                    if deferred:
                        _flush_deferred()
                    nc.sync.dma_start(
                        out=aout[q, :, j - 3:j + 1], in_=seg)

            pend = emit_apr_side(0)
            for t in range(len(tiles)):
                nxt = emit_apr_side(t + 1) if t + 1 < len(tiles) else None
                emit_dpr_side(t, pend)
                pend = nxt

    _split_multi_waits(nc)
    return nc


def kernel(X, W, Z, beta):
    global LAST_RESULT
    import ml_dtypes
    from concourse.bass_utils import run_bass_kernel_spmd

    X = np.asarray(X, dtype=np.float32)
    W = np.asarray(W, dtype=np.float32)
    Z = np.asarray(Z, dtype=np.float32)
    beta_f = float(np.asarray(beta))

    bf16 = ml_dtypes.bfloat16

    # Host: normalized, transposed bf16 operands
    X_emb = X @ W                                            # [N, E] fp32
    Xn = np.sqrt(np.sum(X_emb * X_emb, axis=-1))             # [N]
    Zn = np.sqrt(np.sum(Z * Z, axis=-1))                     # [H, N]
    Xh = X_emb / (Xn[:, None] + EPS)                         # [N, E]
    Zh = Z / (Zn[:, :, None] + EPS)                          # [H, N, E]
    XT = np.ascontiguousarray(Xh.T).astype(bf16)             # [E, N]
    xt_full = XT.reshape(EMB, N_BLK, BLK)

    if "nc" not in _CACHE:
        _CACHE["nc"] = _build_program()
    nc = _CACHE["nc"]

    in_maps = []
    for c in range(N_CORES):
        rows = slice(c * ROWS_PER_CORE, (c + 1) * ROWS_PER_CORE)
        # [E, H, 1024]
        zt_c = np.ascontiguousarray(
            Zh[:, rows, :].transpose(2, 0, 1)
        ).astype(bf16)
        in_maps.append({"xt": xt_full, "zt": zt_c})

    res = None
    for attempt in range(3):
        try:
            res = run_bass_kernel_spmd(nc, in_maps, list(range(N_CORES)))
            break
        except Exception:
            if attempt == 2:
                raise
    LAST_RESULT = res

    # Assemble pooled A: host merges the two shipped planes.
    A = np.empty((N_NODES, N_NODES), dtype=np.float32)
    for c in range(N_CORES):
        a_c = res.results[c]["aout"]  # [8, 128, 16, 2, 512] bf16
        planes = a_c.reshape(ROWS_PER_CORE, N_BLK, 2, BLK)
        rows = slice(c * ROWS_PER_CORE, (c + 1) * ROWS_PER_CORE)
        np.maximum(
            planes[:, :, 0, :].astype(np.float32),
            planes[:, :, 1, :].astype(np.float32),
            out=A[rows].reshape(ROWS_PER_CORE, N_BLK, BLK),
        )

    out = np.empty_like(A)
    B = 1024
    nb = N_NODES // B
    for bi in range(nb):
        ri = slice(bi * B, (bi + 1) * B)
        for bj in range(bi, nb):
            cj = slice(bj * B, (bj + 1) * B)
            S = A[ri, cj] + A[cj, ri].T
            S *= np.float32(0.25)
            S += np.float32(0.5)
            out[ri, cj] = S
            if bj != bi:
                out[cj, ri] = S.T

    if beta_f != 1.0:
        out = np.power(out, beta_f, dtype=np.float32)
    return out


# revision 3
# speedup vs baseline: 1.0932x; 1.0063x over previous
"""Trainium2 Bass kernel for nn_AffNet (affinity network) — v3.

Reference computation:
    X_emb = X @ W                               # [N, E]
    aff_h = (Z_h @ X_emb^T) / (|X_emb| |Z_h|)   # cosine, [H, N, N]
    aff   = max_h aff_h                          # [N, N]
    aff   = (aff + aff^T) / 2                    # symmetrize
    aff   = (aff + 1) / 2                        # [0, 1]
    aff   = aff ** beta

Device strategy (8 NeuronCores, output-row parallel):
  Each core computes 2 of the 16 block-rows of the POOLED (pre-symmetrize)
  affinity A = max_h(Zh_hat @ Xh_hat^T): 8 m-chunk rows x 16 col blocks of
  [128, 512] tiles. The device only pools 4 heads -> 2 half-pooled planes;
  the host finishes with A = max(plane0, plane1), then
  out = 0.25*(A + A^T) + 0.5 and ^beta.

  Rationale (cost model): TensorTensor ops allow at most one PSUM operand,
  the Pool engine supports no two-tensor elementwise op at all, and matmul
  PSUM output is fp32-only on TRN2. So PSUM evacuation (ScalarE copies +
  VectorE 1-PSUM maxes) is the hard floor (~262K free-elem units/core
  across Act+DVE). Shipping two bf16 planes per tile instead of one moves
  the final merge to the host, trading idle DMA bandwidth (~102us, under
  the ~145us engine floor) for the DVE L2 work.

  All operands stay resident in SBUF (Xhat^T replicated 16KB/partition,
  Zhat^T own rows 8KB/partition; one DMA each). Outputs accumulate into a
  [128, 16, 2, 512] ship-stripe per m-chunk row -> 8 output DMAs per core.

  Per tile: 4 matmuls (heads) into one 4-bank PSUM quad (bufs=2 => all 8
  banks). Route mix per tile, statically balanced so Act ~143us, DVE
  ~145us (cost model):
    A2 (116 tiles): ScalarE copies banks {0,1} (fused [128,2,512]);
        VectorE does the fused max(banks {2,3}, e01) straight into the
        ship-stripe. Host max merges (b0,b2) and (b1,b3) planes.
    A4 (12 tiles): ScalarE copies all 4 banks; VectorE does one fused
        bf16 2x max into the stripe.
"""

import numpy as np

N_NODES = 8192
N_FEATURES = 512
EMB = 128
N_HEADS = 4
EPS = 1e-6
N_CORES = 8
BLK = 512
N_BLK = N_NODES // BLK          # 16 col blocks
M_CHUNK = 128
BLOCKS_PER_CORE = 2             # block-rows per core
MROWS = BLOCKS_PER_CORE * (BLK // M_CHUNK)   # 8 m-chunk rows per core
ROWS_PER_CORE = BLOCKS_PER_CORE * BLK        # 1024

_CACHE = {}
LAST_RESULT = None


def _a4h_cols(q):
    """Columns of m-row q on the Act-only A4H route (8 of 128): ScalarE
    evacuates both PSUM pairs (2 planes to the segment, 2 to a side
    buffer), the DVE skips the tile, and the host maxes all 4 planes.
    Rebalances Act ~142us / DVE ~142us without a DVE-blocking burst."""
    return (7, 11) if q == 3 else (7,)


A4H_TILES = [(q, j) for q in range(MROWS) for j in _a4h_cols(q)]


def _split_multi_waits(nc, limit=1):
    """The walrus build in this environment encodes at most one semaphore
    wait per instruction ("Too many sync wait commands" otherwise), while
    Tile attaches several. Hoist extra waits onto same-engine NOPs inserted
    immediately before the instruction (waits still execute before it)."""
    import concourse.mybir as mybir

    for f in nc.m.functions:
        for bb in f.blocks:
            il = bb.instructions  # live list backing the block
            idx = 0
            while idx < len(il):
                inst = il[idx]
                si = inst.sync_info
                waits = list(si.on_wait) if si is not None and si.on_wait else []
                if len(waits) > limit:
                    ups = list(si.on_update) if si.on_update else []
                    inst.sync_info = mybir.SyncInfo(
                        on_wait=waits[:limit], on_update=ups
                    )
                    eng = nc.engines[inst.engine]
                    pos = idx
                    for j in range(limit, len(waits), limit):
                        nbi = eng.nop()
                        ninst = nbi.ins
                        # nop() appended itself to the current bb; detach it
                        removed = False
                        for f2 in nc.m.functions:
                            for bb2 in f2.blocks:
                                l2 = bb2.instructions
                                if l2 and l2[-1].name == ninst.name:
                                    l2.pop()
                                    removed = True
                                    break
                            if removed:
                                break
                        assert removed, "could not detach helper nop"
                        ninst.sync_info = mybir.SyncInfo(
                            on_wait=waits[j : j + limit], on_update=[]
                        )
                        il.insert(pos, ninst)
                        pos += 1
                        idx += 1
                idx += 1


def _build_program():
    import concourse.bass as bass
    import concourse.mybir as mybir
    import concourse.tile as tile

    nc = bass.Bass("TRN2", target_bir_lowering=False, debug=False)

    bf16 = mybir.dt.bfloat16
    f32 = mybir.dt.float32

    # Xhat^T full, [E, 16, 512]; Zhat^T own rows, [E, H, 1024]
    xt = nc.dram_tensor("xt", [EMB, N_BLK, BLK], bf16, kind="ExternalInput")
    zt = nc.dram_tensor("zt", [EMB, N_HEADS, ROWS_PER_CORE], bf16,
                        kind="ExternalInput")
    # two half-pooled planes per tile; host merges
    aout = nc.dram_tensor("aout", [MROWS, M_CHUNK, N_BLK, 2, BLK], bf16,
                          kind="ExternalOutput")
    # extra planes for the A4H tiles: host maxes these in
    bout = nc.dram_tensor("bout", [len(A4H_TILES), M_CHUNK, 2, BLK], bf16,
                          kind="ExternalOutput")

    with tile.TileContext(nc) as tc:
        with (
            tc.tile_pool(name="weights", bufs=1) as wpool,
            tc.tile_pool(name="psum", bufs=1, space="PSUM") as ppool,
            tc.tile_pool(name="work", bufs=2) as spool,
        ):
            xt_s = wpool.tile([EMB, N_BLK, BLK], bf16, name="xt_s")
            zt_s = wpool.tile([EMB, N_HEADS, ROWS_PER_CORE], bf16,
                              name="zt_s")
            # Load order tuned for a fast first tile: the row-0 slices of
            # zt and the first xt block arrive within ~2us, rest streams.
            nc.sync.dma_start(out=zt_s[:, 0:2, 0:M_CHUNK],
                              in_=zt[:, 0:2, 0:M_CHUNK])
            nc.sync.dma_start(out=xt_s[:, 0:1], in_=xt[:, 0:1])
            nc.sync.dma_start(out=zt_s[:, 2:4, 0:M_CHUNK],
                              in_=zt[:, 2:4, 0:M_CHUNK])
            nc.sync.dma_start(out=xt_s[:, 1:3], in_=xt[:, 1:3])
            nc.sync.dma_start(out=xt_s[:, 3:6], in_=xt[:, 3:6])
            nc.sync.dma_start(out=zt_s[:, 0:2, M_CHUNK:],
                              in_=zt[:, 0:2, M_CHUNK:])
            nc.sync.dma_start(out=xt_s[:, 6:11], in_=xt[:, 6:11])
            nc.sync.dma_start(out=zt_s[:, 2:4, M_CHUNK:],
                              in_=zt[:, 2:4, M_CHUNK:])
            nc.sync.dma_start(out=xt_s[:, 11:16], in_=xt[:, 11:16])

            tiles = [(q, j) for q in range(MROWS) for j in range(N_BLK)]
            stripes = {}
            deferred = []

            def get_seg(q, j):
                key = (q, j // 4)
                if key not in stripes:
                    stripes[key] = spool.tile(
                        [M_CHUNK, 4, 2, BLK], bf16,
                        tag="seg", bufs=8, name=f"seg_{q}_{j // 4}")
                return stripes[key]

            def _flush_deferred():
                while deferred:
                    seg_, slot_, a_, b_ = deferred.pop(0)
                    nc.vector.tensor_max(seg_[:, slot_], a_, b_)

            def emit_apr_side(t):
                """Act-pair matmuls + ScalarE evacuation for tile t.
                Emitted one tile AHEAD of the DVE side so the e01 operand
                is ready a full tile before the DVE max needs it. A4H
                tiles evacuate straight into the output segment (slot 3)
                and skip the DVE entirely."""
                q, j = tiles[t]
                msl = slice(q * M_CHUNK, (q + 1) * M_CHUNK)
                rhs = xt_s[:, j]
                apr = ppool.tile([M_CHUNK, 2, BLK], f32, tag="apr",
                                 bufs=2, name=f"apr_{q}_{j}")
                nc.tensor.matmul(apr[:, 0], zt_s[:, 0, msl], rhs,
                                 start=True, stop=True)
                nc.tensor.matmul(apr[:, 1], zt_s[:, 1, msl], rhs,
                                 start=True, stop=True)
                if j in _a4h_cols(q):
                    nc.scalar.copy(get_seg(q, j)[:, j % 4], apr)
                    return None
                e01 = spool.tile([M_CHUNK, 2, BLK], bf16, tag="e01",
                                 bufs=8, name=f"e01_{q}_{j}")
                nc.scalar.copy(e01, apr)
                return e01

            def emit_dpr_side(t, e01):
                q, j = tiles[t]
                msl = slice(q * M_CHUNK, (q + 1) * M_CHUNK)
                rhs = xt_s[:, j]
                seg = get_seg(q, j)
                if j in _a4h_cols(q):
                    # Act-only route: apr's planes went straight to the
                    # segment (apr side); heads {2,3} land in a SECOND
                    # apr-tag tile (Act-paced rotation -- keeping the
                    # dpr rotation DVE-only) and ship via the side
                    # buffer. No DVE work; the host maxes all 4 planes.
                    apr2 = ppool.tile([M_CHUNK, 2, BLK], f32, tag="apr",
                                      bufs=2, name=f"apr2_{q}_{j}")
                    nc.tensor.matmul(apr2[:, 0], zt_s[:, 2, msl], rhs,
                                     start=True, stop=True)
                    nc.tensor.matmul(apr2[:, 1], zt_s[:, 3, msl], rhs,
                                     start=True, stop=True)
                    bseg = spool.tile([M_CHUNK, 2, BLK], bf16, tag="bseg",
                                      bufs=2, name=f"bseg_{q}_{j}")
                    nc.scalar.copy(bseg, apr2)
                    nc.sync.dma_start(out=bout[A4H_TILES.index((q, j))],
                                      in_=bseg)
                else:
                    dpr = ppool.tile([M_CHUNK, 2, BLK], f32, tag="dpr",
                                     bufs=2, name=f"dpr_{q}_{j}")
                    nc.tensor.matmul(dpr[:, 0], zt_s[:, 2, msl], rhs,
                                     start=True, stop=True)
                    nc.tensor.matmul(dpr[:, 1], zt_s[:, 3, msl], rhs,
                                     start=True, stop=True)
                    # Main route: DVE fused max(dve-pair, e01)
                    nc.vector.tensor_max(seg[:, j % 4], dpr, e01)
                    _flush_deferred()
                last_seg = (q == MROWS - 1 and j >= N_BLK - 4)
                if last_seg and j % 4 == 1:
                    nc.sync.dma_start(
                        out=aout[q, :, j - 1:j + 1], in_=seg[:, 0:2])
                elif last_seg and j % 4 == 3:
                    nc.sync.dma_start(
                        out=aout[q, :, j - 1:j + 1], in_=seg[:, 2:4])
                elif j % 4 == 3:
                    if deferred:
                        _flush_deferred()
                    nc.sync.dma_start(
                        out=aout[q, :, j - 3:j + 1], in_=seg)

            # 3-tile apr lookahead: PE banks apr matmuls before each dpr
            # stall, so ScalarE can run far enough ahead to absorb the
            # A4H copy bursts without starving the DVE.
            LOOK = 3
            pend = [emit_apr_side(t) for t in range(min(LOOK, len(tiles)))]
            for t in range(len(tiles)):
                if t + LOOK < len(tiles):
                    pend.append(emit_apr_side(t + LOOK))
                emit_dpr_side(t, pend.pop(0))

    _split_multi_waits(nc)
    return nc


def kernel(X, W, Z, beta):
    global LAST_RESULT
    import ml_dtypes
    from concourse.bass_utils import run_bass_kernel_spmd

    X = np.asarray(X, dtype=np.float32)
    W = np.asarray(W, dtype=np.float32)
    Z = np.asarray(Z, dtype=np.float32)
    beta_f = float(np.asarray(beta))

    bf16 = ml_dtypes.bfloat16

    # Host: normalized, transposed bf16 operands
    X_emb = X @ W                                            # [N, E] fp32
    Xn = np.sqrt(np.sum(X_emb * X_emb, axis=-1))             # [N]
    Zn = np.sqrt(np.sum(Z * Z, axis=-1))                     # [H, N]
    Xh = X_emb / (Xn[:, None] + EPS)                         # [N, E]
    Zh = Z / (Zn[:, :, None] + EPS)                          # [H, N, E]
    XT = np.ascontiguousarray(Xh.T).astype(bf16)             # [E, N]
    xt_full = XT.reshape(EMB, N_BLK, BLK)

    if "nc" not in _CACHE:
        _CACHE["nc"] = _build_program()
    nc = _CACHE["nc"]

    in_maps = []
    for c in range(N_CORES):
        rows = slice(c * ROWS_PER_CORE, (c + 1) * ROWS_PER_CORE)
        # [E, H, 1024]
        zt_c = np.ascontiguousarray(
            Zh[:, rows, :].transpose(2, 0, 1)
        ).astype(bf16)
        in_maps.append({"xt": xt_full, "zt": zt_c})

    res = None
    for attempt in range(3):
        try:
            res = run_bass_kernel_spmd(nc, in_maps, list(range(N_CORES)))
            break
        except Exception:
            if attempt == 2:
                raise
    LAST_RESULT = res

    # Assemble pooled A: host merges the two shipped planes, plus the two
    # extra A4H planes for the j==7 blocks.
    A = np.empty((N_NODES, N_NODES), dtype=np.float32)
    for c in range(N_CORES):
        a_c = res.results[c]["aout"]  # [8, 128, 16, 2, 512] bf16
        b_c = res.results[c]["bout"]  # [n_a4h, 128, 2, 512] bf16
        planes = a_c.reshape(ROWS_PER_CORE, N_BLK, 2, BLK)
        rows = slice(c * ROWS_PER_CORE, (c + 1) * ROWS_PER_CORE)
        Ar = A[rows].reshape(ROWS_PER_CORE, N_BLK, BLK)
        np.maximum(
            planes[:, :, 0, :].astype(np.float32),
            planes[:, :, 1, :].astype(np.float32),
            out=Ar,
        )
        for idx, (q, j) in enumerate(A4H_TILES):
            bmax = np.maximum(
                b_c[idx, :, 0].astype(np.float32),
                b_c[idx, :, 1].astype(np.float32),
            )
            rsl = slice(q * M_CHUNK, (q + 1) * M_CHUNK)
            np.maximum(Ar[rsl, j], bmax, out=Ar[rsl, j])

    out = np.empty_like(A)
    B = 1024
    nb = N_NODES // B
    for bi in range(nb):
        ri = slice(bi * B, (bi + 1) * B)
        for bj in range(bi, nb):
            cj = slice(bj * B, (bj + 1) * B)
            S = A[ri, cj] + A[cj, ri].T
            S *= np.float32(0.25)
            S += np.float32(0.5)
            out[ri, cj] = S
            if bj != bi:
                out[cj, ri] = S.T

    if beta_f != 1.0:
        out = np.power(out, beta_f, dtype=np.float32)
    return out


# revision 5
# speedup vs baseline: 1.1070x; 1.0126x over previous
"""Trainium2 Bass kernel for nn_AffNet (affinity network).

Reference computation:
    X_emb = X @ W                               # [N, E]
    aff_h = (Z_h @ X_emb^T) / (|X_emb| |Z_h|)   # cosine, [H, N, N]
    aff   = max_h aff_h                          # [N, N]
    aff   = (aff + aff^T) / 2                    # symmetrize
    aff   = (aff + 1) / 2                        # [0, 1]
    aff   = aff ** beta

Device strategy (8 NeuronCores, output-row parallel):
  Each core computes 2 of the 16 block-rows of the POOLED (pre-symmetrize)
  affinity A = max_h(Zh_hat @ Xh_hat^T): 8 m-chunk rows x 16 col blocks of
  [128, 512] tiles. The device only pools 4 heads -> 2 half-pooled planes;
  the host finishes with A = max(plane0, plane1), then
  out = 0.25*(A + A^T) + 0.5 and ^beta.

  Rationale (cost model): TensorTensor ops allow at most one PSUM operand,
  the Pool engine supports no two-tensor elementwise op at all, and matmul
  PSUM output is fp32-only on TRN2. So PSUM evacuation (ScalarE copies +
  VectorE 1-PSUM maxes) is the hard floor (~262K free-elem units/core
  across Act+DVE). Shipping two bf16 planes per tile instead of one moves
  the final merge to the host, trading idle DMA bandwidth (~102us, under
  the ~145us engine floor) for the DVE L2 work.

  All operands stay resident in SBUF (Xhat^T replicated 16KB/partition,
  Zhat^T own rows 8KB/partition; one DMA each). Outputs accumulate into a
  [128, 16, 2, 512] ship-stripe per m-chunk row -> 8 output DMAs per core.

  Per tile: 4 matmuls (heads) into two independently rotating 2-bank
  PSUM pairs (apr: heads {0,1} -> ScalarE; dpr: heads {2,3} -> VectorE;
  bufs=2 each => all 8 banks). Static route mix balancing Act ~142us =
  DVE ~142us busy (cost model; ~92% occupancy):
    A2 (119 tiles): ScalarE fused-copies the apr pair to bf16 (e01);
        VectorE does ONE fused max(dpr, e01) [128,2,512] straight into
        the output segment (planes {max(h2,h0), max(h3,h1)}).
    A4H (9 tiles): ScalarE evacuates BOTH pairs (apr -> segment planes,
        heads {2,3} via a second apr-tag tile -> side buffer); the DVE
        skips the tile and the host maxes all 4 planes. Keeping the
        dpr rotation DVE-only avoids cross-engine burst stalls.
  The apr side (matmuls + ScalarE copy) is emitted LOOK=6 tiles ahead of
  the DVE side so ScalarE banks enough lead to absorb A4H bursts.
  Outputs ship as 4-column segments (per-column on the final row) so the
  DMA tail after the last max is short; inputs stream in finely chunked
  DMAs ordered so the first tile's operands land ~4us in.
"""

import numpy as np

N_NODES = 8192
N_FEATURES = 512
EMB = 128
N_HEADS = 4
EPS = 1e-6
N_CORES = 8
BLK = 512
N_BLK = N_NODES // BLK          # 16 col blocks
M_CHUNK = 128
BLOCKS_PER_CORE = 2             # block-rows per core
MROWS = BLOCKS_PER_CORE * (BLK // M_CHUNK)   # 8 m-chunk rows per core
ROWS_PER_CORE = BLOCKS_PER_CORE * BLK        # 1024

_CACHE = {}
LAST_RESULT = None


def _a4h_cols(q):
    """Columns of m-row q on the Act-only A4H route (8 of 128): ScalarE
    evacuates both PSUM pairs (2 planes to the segment, 2 to a side
    buffer), the DVE skips the tile, and the host maxes all 4 planes.
    Rebalances Act ~142us / DVE ~142us without a DVE-blocking burst."""
    return (7, 11) if q == 3 else (7,)


A4H_TILES = [(q, j) for q in range(MROWS) for j in _a4h_cols(q)]


def _split_multi_waits(nc, limit=1):
    """The walrus build in this environment encodes at most one semaphore
    wait per instruction ("Too many sync wait commands" otherwise), while
    Tile attaches several. Hoist extra waits onto same-engine NOPs inserted
    immediately before the instruction (waits still execute before it)."""
    import concourse.mybir as mybir

    for f in nc.m.functions:
        for bb in f.blocks:
            il = bb.instructions  # live list backing the block
            idx = 0
            while idx < len(il):
                inst = il[idx]
                si = inst.sync_info
                waits = list(si.on_wait) if si is not None and si.on_wait else []
                if len(waits) > limit:
                    ups = list(si.on_update) if si.on_update else []
                    inst.sync_info = mybir.SyncInfo(
                        on_wait=waits[:limit], on_update=ups
                    )
                    eng = nc.engines[inst.engine]
                    pos = idx
                    for j in range(limit, len(waits), limit):
                        nbi = eng.nop()
                        ninst = nbi.ins
                        # nop() appended itself to the current bb; detach it
                        removed = False
                        for f2 in nc.m.functions:
                            for bb2 in f2.blocks:
                                l2 = bb2.instructions
                                if l2 and l2[-1].name == ninst.name:
                                    l2.pop()
                                    removed = True
                                    break
                            if removed:
                                break
                        assert removed, "could not detach helper nop"
                        ninst.sync_info = mybir.SyncInfo(
                            on_wait=waits[j : j + limit], on_update=[]
                        )
                        il.insert(pos, ninst)
                        pos += 1
                        idx += 1
                idx += 1


def _build_program():
    import concourse.bass as bass
    import concourse.mybir as mybir
    import concourse.tile as tile

    nc = bass.Bass("TRN2", target_bir_lowering=False, debug=False)

    bf16 = mybir.dt.bfloat16
    f32 = mybir.dt.float32

    # Xhat^T full, [E, 16, 512]; Zhat^T own rows, [E, H, 1024]
    xt = nc.dram_tensor("xt", [EMB, N_BLK, BLK], bf16, kind="ExternalInput")
    zt = nc.dram_tensor("zt", [EMB, N_HEADS, ROWS_PER_CORE], bf16,
                        kind="ExternalInput")
    # two half-pooled planes per tile; host merges
    aout = nc.dram_tensor("aout", [MROWS, M_CHUNK, N_BLK, 2, BLK], bf16,
                          kind="ExternalOutput")
    # extra planes for the A4H tiles: host maxes these in
    bout = nc.dram_tensor("bout", [len(A4H_TILES), M_CHUNK, 2, BLK], bf16,
                          kind="ExternalOutput")

    with tile.TileContext(nc) as tc:
        with (
            tc.tile_pool(name="weights", bufs=1) as wpool,
            tc.tile_pool(name="psum", bufs=1, space="PSUM") as ppool,
            tc.tile_pool(name="work", bufs=2) as spool,
        ):
            xt_s = wpool.tile([EMB, N_BLK, BLK], bf16, name="xt_s")
            zt_s = wpool.tile([EMB, N_HEADS, ROWS_PER_CORE], bf16,
                              name="zt_s")
            # Load order tuned for a fast first tile: the row-0 slices of
            # zt and the first xt block arrive within ~2us, rest streams.
            nc.sync.dma_start(out=zt_s[:, 0:2, 0:M_CHUNK],
                              in_=zt[:, 0:2, 0:M_CHUNK])
            nc.sync.dma_start(out=xt_s[:, 0:1], in_=xt[:, 0:1])
            nc.sync.dma_start(out=zt_s[:, 2:4, 0:M_CHUNK],
                              in_=zt[:, 2:4, 0:M_CHUNK])
            nc.sync.dma_start(out=xt_s[:, 1:3], in_=xt[:, 1:3])
            nc.sync.dma_start(out=xt_s[:, 3:6], in_=xt[:, 3:6])
            nc.sync.dma_start(out=zt_s[:, 0:2, M_CHUNK:],
                              in_=zt[:, 0:2, M_CHUNK:])
            nc.sync.dma_start(out=xt_s[:, 6:11], in_=xt[:, 6:11])
            nc.sync.dma_start(out=zt_s[:, 2:4, M_CHUNK:],
                              in_=zt[:, 2:4, M_CHUNK:])
            nc.sync.dma_start(out=xt_s[:, 11:16], in_=xt[:, 11:16])

            tiles = [(q, j) for q in range(MROWS) for j in range(N_BLK)]
            stripes = {}
            deferred = []

            def get_seg(q, j):
                key = (q, j // 4)
                if key not in stripes:
                    stripes[key] = spool.tile(
                        [M_CHUNK, 4, 2, BLK], bf16,
                        tag="seg", bufs=8, name=f"seg_{q}_{j // 4}")
                return stripes[key]

            def _flush_deferred():
                while deferred:
                    seg_, slot_, a_, b_ = deferred.pop(0)
                    nc.vector.tensor_max(seg_[:, slot_], a_, b_)

            def emit_apr_side(t):
                """Act-pair matmuls + ScalarE evacuation for tile t.
                Emitted one tile AHEAD of the DVE side so the e01 operand
                is ready a full tile before the DVE max needs it. A4H
                tiles evacuate straight into the output segment (slot 3)
                and skip the DVE entirely."""
                q, j = tiles[t]
                msl = slice(q * M_CHUNK, (q + 1) * M_CHUNK)
                rhs = xt_s[:, j]
                apr = ppool.tile([M_CHUNK, 2, BLK], f32, tag="apr",
                                 bufs=2, name=f"apr_{q}_{j}")
                nc.tensor.matmul(apr[:, 0], zt_s[:, 0, msl], rhs,
                                 start=True, stop=True)
                nc.tensor.matmul(apr[:, 1], zt_s[:, 1, msl], rhs,
                                 start=True, stop=True)
                if j in _a4h_cols(q):
                    nc.scalar.copy(get_seg(q, j)[:, j % 4], apr)
                    return None
                e01 = spool.tile([M_CHUNK, 2, BLK], bf16, tag="e01",
                                 bufs=10, name=f"e01_{q}_{j}")
                nc.scalar.copy(e01, apr)
                return e01

            def emit_dpr_side(t, e01):
                q, j = tiles[t]
                msl = slice(q * M_CHUNK, (q + 1) * M_CHUNK)
                rhs = xt_s[:, j]
                seg = get_seg(q, j)
                if j in _a4h_cols(q):
                    # Act-only route: apr's planes went straight to the
                    # segment (apr side); heads {2,3} land in a SECOND
                    # apr-tag tile (Act-paced rotation -- keeping the
                    # dpr rotation DVE-only) and ship via the side
                    # buffer. No DVE work; the host maxes all 4 planes.
                    apr2 = ppool.tile([M_CHUNK, 2, BLK], f32, tag="apr",
                                      bufs=2, name=f"apr2_{q}_{j}")
                    nc.tensor.matmul(apr2[:, 0], zt_s[:, 2, msl], rhs,
                                     start=True, stop=True)
                    nc.tensor.matmul(apr2[:, 1], zt_s[:, 3, msl], rhs,
                                     start=True, stop=True)
                    bseg = spool.tile([M_CHUNK, 2, BLK], bf16, tag="bseg",
                                      bufs=2, name=f"bseg_{q}_{j}")
                    nc.scalar.copy(bseg, apr2)
                    nc.sync.dma_start(out=bout[A4H_TILES.index((q, j))],
                                      in_=bseg)
                else:
                    dpr = ppool.tile([M_CHUNK, 2, BLK], f32, tag="dpr",
                                     bufs=2, name=f"dpr_{q}_{j}")
                    nc.tensor.matmul(dpr[:, 0], zt_s[:, 2, msl], rhs,
                                     start=True, stop=True)
                    nc.tensor.matmul(dpr[:, 1], zt_s[:, 3, msl], rhs,
                                     start=True, stop=True)
                    # Main route: DVE fused max(dve-pair, e01)
                    nc.vector.tensor_max(seg[:, j % 4], dpr, e01)
                    _flush_deferred()
                if q == MROWS - 1 and j >= N_BLK - 8:
                    # tail: per-column shipping so the final DMA is short
                    nc.sync.dma_start(
                        out=aout[q, :, j:j + 1], in_=seg[:, j % 4:j % 4 + 1])
                elif j % 4 == 3:
                    if deferred:
                        _flush_deferred()
                    nc.sync.dma_start(
                        out=aout[q, :, j - 3:j + 1], in_=seg)

            # 3-tile apr lookahead: PE banks apr matmuls before each dpr
            # stall, so ScalarE can run far enough ahead to absorb the
            # A4H copy bursts without starving the DVE.
            LOOK = 6
            pend = [emit_apr_side(t) for t in range(min(LOOK, len(tiles)))]
            for t in range(len(tiles)):
                if t + LOOK < len(tiles):
                    pend.append(emit_apr_side(t + LOOK))
                emit_dpr_side(t, pend.pop(0))

    _split_multi_waits(nc)
    return nc


def kernel(X, W, Z, beta):
    global LAST_RESULT
    import ml_dtypes
    from concourse.bass_utils import run_bass_kernel_spmd

    X = np.asarray(X, dtype=np.float32)
    W = np.asarray(W, dtype=np.float32)
    Z = np.asarray(Z, dtype=np.float32)
    beta_f = float(np.asarray(beta))

    bf16 = ml_dtypes.bfloat16

    # Host: normalized, transposed bf16 operands
    X_emb = X @ W                                            # [N, E] fp32
    Xn = np.sqrt(np.sum(X_emb * X_emb, axis=-1))             # [N]
    Zn = np.sqrt(np.sum(Z * Z, axis=-1))                     # [H, N]
    Xh = X_emb / (Xn[:, None] + EPS)                         # [N, E]
    Zh = Z / (Zn[:, :, None] + EPS)                          # [H, N, E]
    XT = np.ascontiguousarray(Xh.T).astype(bf16)             # [E, N]
    xt_full = XT.reshape(EMB, N_BLK, BLK)

    if "nc" not in _CACHE:
        _CACHE["nc"] = _build_program()
    nc = _CACHE["nc"]

    in_maps = []
    for c in range(N_CORES):
        rows = slice(c * ROWS_PER_CORE, (c + 1) * ROWS_PER_CORE)
        # [E, H, 1024]
        zt_c = np.ascontiguousarray(
            Zh[:, rows, :].transpose(2, 0, 1)
        ).astype(bf16)
        in_maps.append({"xt": xt_full, "zt": zt_c})

    res = None
    for attempt in range(3):
        try:
            res = run_bass_kernel_spmd(nc, in_maps, list(range(N_CORES)))
            break
        except Exception:
            if attempt == 2:
                raise
    LAST_RESULT = res

    # Assemble pooled A: host merges the two shipped planes, plus the two
    # extra A4H planes for the j==7 blocks.
    A = np.empty((N_NODES, N_NODES), dtype=np.float32)
    for c in range(N_CORES):
        a_c = res.results[c]["aout"]  # [8, 128, 16, 2, 512] bf16
        b_c = res.results[c]["bout"]  # [n_a4h, 128, 2, 512] bf16
        planes = a_c.reshape(ROWS_PER_CORE, N_BLK, 2, BLK)
        rows = slice(c * ROWS_PER_CORE, (c + 1) * ROWS_PER_CORE)
        Ar = A[rows].reshape(ROWS_PER_CORE, N_BLK, BLK)
        np.maximum(
            planes[:, :, 0, :].astype(np.float32),
            planes[:, :, 1, :].astype(np.float32),
            out=Ar,
        )
        for idx, (q, j) in enumerate(A4H_TILES):
            bmax = np.maximum(
                b_c[idx, :, 0].astype(np.float32),
                b_c[idx, :, 1].astype(np.float32),
            )
            rsl = slice(q * M_CHUNK, (q + 1) * M_CHUNK)
            np.maximum(Ar[rsl, j], bmax, out=Ar[rsl, j])

    out = np.empty_like(A)
    B = 1024
    nb = N_NODES // B
    for bi in range(nb):
        ri = slice(bi * B, (bi + 1) * B)
        for bj in range(bi, nb):
            cj = slice(bj * B, (bj + 1) * B)
            S = A[ri, cj] + A[cj, ri].T
            S *= np.float32(0.25)
            S += np.float32(0.5)
            out[ri, cj] = S
            if bj != bi:
                out[cj, ri] = S.T

    if beta_f != 1.0:
        out = np.power(out, beta_f, dtype=np.float32)
    return out


# revision 6
# speedup vs baseline: 1.1084x; 1.0013x over previous
"""Trainium2 Bass kernel for nn_AffNet (affinity network).

Reference computation:
    X_emb = X @ W                               # [N, E]
    aff_h = (Z_h @ X_emb^T) / (|X_emb| |Z_h|)   # cosine, [H, N, N]
    aff   = max_h aff_h                          # [N, N]
    aff   = (aff + aff^T) / 2                    # symmetrize
    aff   = (aff + 1) / 2                        # [0, 1]
    aff   = aff ** beta

Device strategy (8 NeuronCores, output-row parallel):
  Each core computes 2 of the 16 block-rows of the POOLED (pre-symmetrize)
  affinity A = max_h(Zh_hat @ Xh_hat^T): 8 m-chunk rows x 16 col blocks of
  [128, 512] tiles. The device only pools 4 heads -> 2 half-pooled planes;
  the host finishes with A = max(plane0, plane1), then
  out = 0.25*(A + A^T) + 0.5 and ^beta.

  Rationale (cost model): TensorTensor ops allow at most one PSUM operand,
  the Pool engine supports no two-tensor elementwise op at all, and matmul
  PSUM output is fp32-only on TRN2. So PSUM evacuation (ScalarE copies +
  VectorE 1-PSUM maxes) is the hard floor (~262K free-elem units/core
  across Act+DVE). Shipping two bf16 planes per tile instead of one moves
  the final merge to the host, trading idle DMA bandwidth (~102us, under
  the ~145us engine floor) for the DVE L2 work.

  All operands stay resident in SBUF (Xhat^T replicated 16KB/partition,
  Zhat^T own rows 8KB/partition; one DMA each). Outputs accumulate into a
  [128, 16, 2, 512] ship-stripe per m-chunk row -> 8 output DMAs per core.

  Per tile: 4 matmuls (heads) into two independently rotating 2-bank
  PSUM pairs (apr: heads {0,1} -> ScalarE; dpr: heads {2,3} -> VectorE;
  bufs=2 each => all 8 banks). Static route mix balancing Act ~142us =
  DVE ~142us busy (cost model; ~92% occupancy):
    A2 (119 tiles): ScalarE fused-copies the apr pair to bf16 (e01);
        VectorE does ONE fused max(dpr, e01) [128,2,512] straight into
        the output segment (planes {max(h2,h0), max(h3,h1)}).
    A4H (9 tiles): ScalarE evacuates BOTH pairs (apr -> segment planes,
        heads {2,3} via a second apr-tag tile -> side buffer); the DVE
        skips the tile and the host maxes all 4 planes. Keeping the
        dpr rotation DVE-only avoids cross-engine burst stalls.
  The apr side (matmuls + ScalarE copy) is emitted LOOK=6 tiles ahead of
  the DVE side so ScalarE banks enough lead to absorb A4H bursts.
  Outputs ship as 4-column segments (per-column on the final row) so the
  DMA tail after the last max is short; inputs stream in finely chunked
  DMAs ordered so the first tile's operands land ~4us in.
"""

import numpy as np

N_NODES = 8192
N_FEATURES = 512
EMB = 128
N_HEADS = 4
EPS = 1e-6
N_CORES = 8
BLK = 512
N_BLK = N_NODES // BLK          # 16 col blocks
M_CHUNK = 128
BLOCKS_PER_CORE = 2             # block-rows per core
MROWS = BLOCKS_PER_CORE * (BLK // M_CHUNK)   # 8 m-chunk rows per core
ROWS_PER_CORE = BLOCKS_PER_CORE * BLK        # 1024

_CACHE = {}
LAST_RESULT = None


def _a4h_cols(q):
    """Columns of m-row q on the Act-only A4H route (8 of 128): ScalarE
    evacuates both PSUM pairs (2 planes to the segment, 2 to a side
    buffer), the DVE skips the tile, and the host maxes all 4 planes.
    Rebalances Act ~142us / DVE ~142us without a DVE-blocking burst."""
    return (7, 11) if q == 3 else (7,)


A4H_TILES = [(q, j) for q in range(MROWS) for j in _a4h_cols(q)]


def _split_multi_waits(nc, limit=1):
    """The walrus build in this environment encodes at most one semaphore
    wait per instruction ("Too many sync wait commands" otherwise), while
    Tile attaches several. Hoist extra waits onto same-engine NOPs inserted
    immediately before the instruction (waits still execute before it)."""
    import concourse.mybir as mybir

    for f in nc.m.functions:
        for bb in f.blocks:
            il = bb.instructions  # live list backing the block
            idx = 0
            while idx < len(il):
                inst = il[idx]
                si = inst.sync_info
                waits = list(si.on_wait) if si is not None and si.on_wait else []
                if len(waits) > limit:
                    ups = list(si.on_update) if si.on_update else []
                    inst.sync_info = mybir.SyncInfo(
                        on_wait=waits[:limit], on_update=ups
                    )
                    eng = nc.engines[inst.engine]
                    pos = idx
                    for j in range(limit, len(waits), limit):
                        nbi = eng.nop()
                        ninst = nbi.ins
                        # nop() appended itself to the current bb; detach it
                        removed = False
                        for f2 in nc.m.functions:
                            for bb2 in f2.blocks:
                                l2 = bb2.instructions
                                if l2 and l2[-1].name == ninst.name:
                                    l2.pop()
                                    removed = True
                                    break
                            if removed:
                                break
                        assert removed, "could not detach helper nop"
                        ninst.sync_info = mybir.SyncInfo(
                            on_wait=waits[j : j + limit], on_update=[]
                        )
                        il.insert(pos, ninst)
                        pos += 1
                        idx += 1
                idx += 1


def _build_program():
    import concourse.bass as bass
    import concourse.mybir as mybir
    import concourse.tile as tile

    nc = bass.Bass("TRN2", target_bir_lowering=False, debug=False)

    bf16 = mybir.dt.bfloat16
    f32 = mybir.dt.float32

    # Xhat^T full, [E, 16, 512]; Zhat^T own rows, [E, H, 1024]
    xt = nc.dram_tensor("xt", [EMB, N_BLK, BLK], bf16, kind="ExternalInput")
    zt = nc.dram_tensor("zt", [EMB, N_HEADS, ROWS_PER_CORE], bf16,
                        kind="ExternalInput")
    fp8 = mybir.dt.float8e4
    # three planes per tile (2 raw from ScalarE, 1 pooled from VectorE)
    aout = nc.dram_tensor("aout", [MROWS, M_CHUNK, N_BLK, 3, BLK], fp8,
                          kind="ExternalOutput")
    # extra planes for the A4H tiles: host maxes these in
    bout = nc.dram_tensor("bout", [len(A4H_TILES), M_CHUNK, 2, BLK], fp8,
                          kind="ExternalOutput")

    with tile.TileContext(nc) as tc:
        with (
            tc.tile_pool(name="weights", bufs=1) as wpool,
            tc.tile_pool(name="psum", bufs=1, space="PSUM") as ppool,
            tc.tile_pool(name="work", bufs=2) as spool,
        ):
            xt_s = wpool.tile([EMB, N_BLK, BLK], bf16, name="xt_s")
            zt_s = wpool.tile([EMB, N_HEADS, ROWS_PER_CORE], bf16,
                              name="zt_s")
            # Load order tuned for a fast first tile: the row-0 slices of
            # zt and the first xt block arrive within ~2us, rest streams.
            nc.sync.dma_start(out=zt_s[:, 0:2, 0:M_CHUNK],
                              in_=zt[:, 0:2, 0:M_CHUNK])
            nc.sync.dma_start(out=xt_s[:, 0:1], in_=xt[:, 0:1])
            nc.sync.dma_start(out=zt_s[:, 2:4, 0:M_CHUNK],
                              in_=zt[:, 2:4, 0:M_CHUNK])
            nc.sync.dma_start(out=xt_s[:, 1:3], in_=xt[:, 1:3])
            nc.sync.dma_start(out=xt_s[:, 3:6], in_=xt[:, 3:6])
            nc.sync.dma_start(out=zt_s[:, 0:2, M_CHUNK:],
                              in_=zt[:, 0:2, M_CHUNK:])
            nc.sync.dma_start(out=xt_s[:, 6:11], in_=xt[:, 6:11])
            nc.sync.dma_start(out=zt_s[:, 2:4, M_CHUNK:],
                              in_=zt[:, 2:4, M_CHUNK:])
            nc.sync.dma_start(out=xt_s[:, 11:16], in_=xt[:, 11:16])

            tiles = [(q, j) for q in range(MROWS) for j in range(N_BLK)]
            stripes = {}
            deferred = []

            def get_seg(q, j):
                key = (q, j // 4)
                if key not in stripes:
                    stripes[key] = spool.tile(
                        [M_CHUNK, 4, 3, BLK], fp8,
                        tag="seg", bufs=8, name=f"seg_{q}_{j // 4}")
                return stripes[key]

            def _flush_deferred():
                while deferred:
                    seg_, slot_, a_, b_ = deferred.pop(0)
                    nc.vector.tensor_max(seg_[:, slot_], a_, b_)

            def emit_apr_side(t):
                """Act-pair matmuls + ScalarE evacuation for tile t.
                Emitted one tile AHEAD of the DVE side so the e01 operand
                is ready a full tile before the DVE max needs it. A4H
                tiles evacuate straight into the output segment (slot 3)
                and skip the DVE entirely."""
                q, j = tiles[t]
                msl = slice(q * M_CHUNK, (q + 1) * M_CHUNK)
                rhs = xt_s[:, j]
                apr = ppool.tile([M_CHUNK, 2, BLK], f32, tag="apr",
                                 bufs=2, name=f"apr_{q}_{j}")
                nc.tensor.matmul(apr[:, 0], zt_s[:, 0, msl], rhs,
                                 start=True, stop=True)
                nc.tensor.matmul(apr[:, 1], zt_s[:, 1, msl], rhs,
                                 start=True, stop=True)
                nc.scalar.copy(get_seg(q, j)[:, j % 4, 0:2], apr)
                return None

            def emit_dpr_side(t, e01):
                q, j = tiles[t]
                msl = slice(q * M_CHUNK, (q + 1) * M_CHUNK)
                rhs = xt_s[:, j]
                seg = get_seg(q, j)
                if j in _a4h_cols(q):
                    # Act-only route: heads {2,3} land in a SECOND
                    # apr-tag tile (Act-paced rotation -- keeping the
                    # dpr rotation DVE-only) and ship via the side
                    # buffer; the idle Pool engine pads the pooled
                    # plane with -448 so the host can max all planes.
                    apr2 = ppool.tile([M_CHUNK, 2, BLK], f32, tag="apr",
                                      bufs=2, name=f"apr2_{q}_{j}")
                    nc.tensor.matmul(apr2[:, 0], zt_s[:, 2, msl], rhs,
                                     start=True, stop=True)
                    nc.tensor.matmul(apr2[:, 1], zt_s[:, 3, msl], rhs,
                                     start=True, stop=True)
                    bseg = spool.tile([M_CHUNK, 2, BLK], fp8, tag="bseg",
                                      bufs=2, name=f"bseg_{q}_{j}")
                    nc.scalar.copy(bseg, apr2)
                    nc.sync.dma_start(out=bout[A4H_TILES.index((q, j))],
                                      in_=bseg)
                    nc.gpsimd.memset(seg[:, j % 4, 2], -448.0)
                else:
                    dpr = ppool.tile([M_CHUNK, 2, BLK], f32, tag="dpr",
                                     bufs=2, name=f"dpr_{q}_{j}")
                    nc.tensor.matmul(dpr[:, 0], zt_s[:, 2, msl], rhs,
                                     start=True, stop=True)
                    nc.tensor.matmul(dpr[:, 1], zt_s[:, 3, msl], rhs,
                                     start=True, stop=True)
                    # Main route: DVE pools its pair independently
                    # (no ScalarE handoff at all)
                    nc.vector.tensor_reduce(
                        seg[:, j % 4, 2], dpr.transpose([0, 2, 1]),
                        axis=mybir.AxisListType.X, op=mybir.AluOpType.max)
                    _flush_deferred()
                if q == MROWS - 1 and j >= N_BLK - 8:
                    # tail: per-column shipping so the final DMA is short
                    nc.sync.dma_start(
                        out=aout[q, :, j:j + 1], in_=seg[:, j % 4:j % 4 + 1])
                elif j % 4 == 3:
                    if deferred:
                        _flush_deferred()
                    nc.sync.dma_start(
                        out=aout[q, :, j - 3:j + 1], in_=seg)

            # 3-tile apr lookahead: PE banks apr matmuls before each dpr
            # stall, so ScalarE can run far enough ahead to absorb the
            # A4H copy bursts without starving the DVE.
            LOOK = 6
            pend = [emit_apr_side(t) for t in range(min(LOOK, len(tiles)))]
            for t in range(len(tiles)):
                if t + LOOK < len(tiles):
                    pend.append(emit_apr_side(t + LOOK))
                emit_dpr_side(t, pend.pop(0))

    _split_multi_waits(nc)
    return nc


def kernel(X, W, Z, beta):
    global LAST_RESULT
    import ml_dtypes
    from concourse.bass_utils import run_bass_kernel_spmd

    X = np.asarray(X, dtype=np.float32)
    W = np.asarray(W, dtype=np.float32)
    Z = np.asarray(Z, dtype=np.float32)
    beta_f = float(np.asarray(beta))

    bf16 = ml_dtypes.bfloat16

    # Host: normalized, transposed bf16 operands
    X_emb = X @ W                                            # [N, E] fp32
    Xn = np.sqrt(np.sum(X_emb * X_emb, axis=-1))             # [N]
    Zn = np.sqrt(np.sum(Z * Z, axis=-1))                     # [H, N]
    Xh = X_emb / (Xn[:, None] + EPS)                         # [N, E]
    Zh = Z / (Zn[:, :, None] + EPS)                          # [H, N, E]
    XT = np.ascontiguousarray(Xh.T).astype(bf16)             # [E, N]
    xt_full = XT.reshape(EMB, N_BLK, BLK)

    if "nc" not in _CACHE:
        _CACHE["nc"] = _build_program()
    nc = _CACHE["nc"]

    in_maps = []
    for c in range(N_CORES):
        rows = slice(c * ROWS_PER_CORE, (c + 1) * ROWS_PER_CORE)
        # [E, H, 1024]
        zt_c = np.ascontiguousarray(
            Zh[:, rows, :].transpose(2, 0, 1)
        ).astype(bf16)
        in_maps.append({"xt": xt_full, "zt": zt_c})

    res = None
    for attempt in range(3):
        try:
            res = run_bass_kernel_spmd(nc, in_maps, list(range(N_CORES)))
            break
        except Exception:
            if attempt == 2:
                raise
    LAST_RESULT = res

    # Assemble pooled A: host merges the two shipped planes, plus the two
    # extra A4H planes for the j==7 blocks.
    A = np.empty((N_NODES, N_NODES), dtype=np.float32)
    for c in range(N_CORES):
        a_c = res.results[c]["aout"]  # [8, 128, 16, 3, 512] fp8
        b_c = res.results[c]["bout"]  # [n_a4h, 128, 2, 512] fp8
        planes = a_c.reshape(ROWS_PER_CORE, N_BLK, 3, BLK)
        rows = slice(c * ROWS_PER_CORE, (c + 1) * ROWS_PER_CORE)
        Ar = A[rows].reshape(ROWS_PER_CORE, N_BLK, BLK)
        np.max(planes.astype(np.float32), axis=2, out=Ar)
        for idx, (q, j) in enumerate(A4H_TILES):
            bmax = np.maximum(
                b_c[idx, :, 0].astype(np.float32),
                b_c[idx, :, 1].astype(np.float32),
            )
            rsl = slice(q * M_CHUNK, (q + 1) * M_CHUNK)
            np.maximum(Ar[rsl, j], bmax, out=Ar[rsl, j])

    out = np.empty_like(A)
    B = 1024
    nb = N_NODES // B
    for bi in range(nb):
        ri = slice(bi * B, (bi + 1) * B)
        for bj in range(bi, nb):
            cj = slice(bj * B, (bj + 1) * B)
            S = A[ri, cj] + A[cj, ri].T
            S *= np.float32(0.25)
            S += np.float32(0.5)
            out[ri, cj] = S
            if bj != bi:
                out[cj, ri] = S.T

    if beta_f != 1.0:
        out = np.power(out, beta_f, dtype=np.float32)
    return out
